# revision 13
# baseline (speedup 1.0000x reference)
"""BiLSTM-CRF NLL kernel for 8 TRN2 NeuronCores.

Sharding: data-parallel over batch. B=128 split into 8 shards of 16
sentences; each core runs both LSTM directions, the fc projection (fused
per-step), the CRF forward pass (exp-domain, renorm every R=8 steps,
capture-at-length), and the gold-path score for its shard.

Recurrence design (per core, per step t, per direction d):
  gates^T [4H=2048, B=16] live in one PSUM tile ps [128, 256], col = m*16+b,
  gate row order permuted to [i | f | o | g]; g rows of W/b pre-scaled by -2
  so tanh(g) = 1 - 2*sigmoid(-2g) comes out of the same sigmoid op.
  Accumulation per m-window: bias (one-hot rhs matmul, start=True) then
  2 k-tiles of Wih @ emb(x_t) (embT resident in SBUF, host-gathered) then
  4 k-tiles of Whh @ h (stop=True).
  Elementwise: 1 sigmoid (Act, 256w) -> gm1 (DVE) -> t2 (DVE), cf (Pool),
  c (DVE, predicated for bwd), tanh_c (Act), h (DVE, bf16 -> hcur slot).
  fc fused: per-step matmuls accumulate emissions^T [12, 512] per 32-step
  chunk in PSUM; first-finishing direction copies to emisT, other adds.
CRF: alpha'_{t+1} = (E @ alpha'_t) * exp(emis_t + fc_b), E = exp(trans)^T;
  renorm every R=8; alpha history in SBUF; per-sentence capture at len-1.
"""

import os
import numpy as np
import ml_dtypes

import concourse.bass as bass
import concourse.bacc as bacc
import concourse.mybir as mybir
import concourse.tile as tile
from concourse.bass import AP

F32 = mybir.dt.float32
BF16 = mybir.dt.bfloat16
I32 = mybir.dt.int32
U8 = mybir.dt.uint8
MUL = mybir.AluOpType.mult
ADD = mybir.AluOpType.add
SUB = mybir.AluOpType.subtract
X = mybir.AxisListType.X
SIG = mybir.ActivationFunctionType.Sigmoid
TANH = mybir.ActivationFunctionType.Tanh

P = 128
B = 16            # batch per core
H = 512
E = 256
G = 2048          # 4H
K = 12
START, STOP = 10, 11
R = 8             # CRF renorm period
NCORES = 8

T = int(os.environ.get("BASS_LSTM_T", "256"))
SKIP = set(os.environ.get("BASS_SKIP", "").split(","))
NE = T // R
NCK = T // 32     # fc chunks


def fv(t, off, pat):
    """Free-dim view of a contiguous [P, F] tile: keep partition pair, replace
    free dims with `pat` (list of [step, count]) at element offset `off`."""
    base = t[:] if not isinstance(t, AP) else t
    part = list(base.ap[0])
    return AP(base.tensor, base.offset + off, [part] + [list(p) for p in pat])


def build(nc):
    dirs = ("f", "b")
    dt = {}

    def din(name, shape, dtype):
        dt[name] = nc.dram_tensor(name, shape, dtype, kind="ExternalInput")
        return dt[name]

    for d in dirs:
        din(f"wihT_{d}", [E, G], BF16)
        din(f"whhT_{d}", [H, G], BF16)
        din(f"bias16_{d}", [16, P], BF16)
        din(f"h0T_{d}", [P, 64], BF16)
        din(f"c0T_{d}", [P, 64], F32)
        din(f"fcWT_{d}", [H, K], BF16)
    din("xembT", [E, T * B], BF16)
    din("onehot16", [16, 256], BF16)
    din("mask_b", [T, P, 64], U8)
    din("transT", [K, K], F32)
    din("trans", [K, K], F32)
    din("fcb", [K], F32)
    din("a0", [K, B], F32)
    din("msel", [K, T * B], F32)
    din("maskep", [NE * B], F32)
    din("sel", [K, T * B], F32)
    din("counts", [B, 144], F32)
    din("cntb", [B, K], F32)

    nll_o = nc.dram_tensor("nll", [B], F32, kind="ExternalOutput")
    demis_o = nc.dram_tensor("dbg_emis", [K, T * B], F32, kind="ExternalOutput")
    dlogz_o = nc.dram_tensor("dbg_logz", [B], F32, kind="ExternalOutput")
    dgold_o = nc.dram_tensor("dbg_gold", [B], F32, kind="ExternalOutput")

    scr16 = nc.dram_tensor("scr16", [B], F32)
    DBG0 = os.environ.get("BASS_DBG0") == "1"
    if DBG0:
        dsg_o = {d: nc.dram_tensor(f"dbg_sg_{d}", [P, 256], F32,
                                   kind="ExternalOutput") for d in dirs}
        dh_o = {d: nc.dram_tensor(f"dbg_h_{d}", [P, 64], BF16,
                                  kind="ExternalOutput") for d in dirs}
        dc_o = {d: nc.dram_tensor(f"dbg_c_{d}", [P, 64], F32,
                                  kind="ExternalOutput") for d in dirs}
        dps_o = {d: nc.dram_tensor(f"dbg_ps_{d}", [P, 256], F32,
                                   kind="ExternalOutput") for d in dirs}

    with tile.TileContext(nc) as tc:
        with tc.tile_pool(name="persist", bufs=1) as pp:
            whh = {d: pp.tile([P, 4 * 16 * P], BF16, name=f"whh{d}", tag=f"whh{d}") for d in dirs}
            wih = {d: pp.tile([P, 2 * 16 * P], BF16, name=f"wih{d}", tag=f"wih{d}") for d in dirs}
            b16 = {d: pp.tile([16, P], BF16, name=f"b16{d}", tag=f"b16{d}") for d in dirs}
            fcw = {d: pp.tile([P, 4 * K], BF16, name=f"fcw{d}", tag=f"fcw{d}") for d in dirs}
            hcur = {d: pp.tile([P, 2 * 64], BF16, name=f"hcur{d}", tag=f"hcur{d}") for d in dirs}
            cst = {d: pp.tile([P, 64], F32, name=f"cst{d}", tag=f"c{d}") for d in dirs}
            oh16 = pp.tile([16, 256], BF16, tag="oh16")
            embT = pp.tile([P, 2 * T * B], BF16, tag="embT")
            emisT = pp.tile([K, T * B], F32, tag="emisT")

            for d in dirs:
                for k in range(4):
                    nc.sync.dma_start(
                        whh[d][:, k * 16 * P:(k + 1) * 16 * P],
                        dt[f"whhT_{d}"].ap()[k * P:(k + 1) * P, :])
                    nc.sync.dma_start(
                        fcw[d][:, k * K:(k + 1) * K],
                        dt[f"fcWT_{d}"].ap()[k * P:(k + 1) * P, :])
                for k in range(2):
                    nc.sync.dma_start(
                        wih[d][:, k * 16 * P:(k + 1) * 16 * P],
                        dt[f"wihT_{d}"].ap()[k * P:(k + 1) * P, :])
                nc.sync.dma_start(b16[d][:], dt[f"bias16_{d}"].ap()[:])
                nc.sync.dma_start(hcur[d][:, 0:64], dt[f"h0T_{d}"].ap()[:])
                nc.sync.dma_start(cst[d][:], dt[f"c0T_{d}"].ap()[:])
            nc.sync.dma_start(oh16[:], dt["onehot16"].ap()[:])
            for k in range(2):
                nc.sync.dma_start(embT[:, k * T * B:(k + 1) * T * B],
                                  dt["xembT"].ap()[k * P:(k + 1) * P, :])

            # ---- recurrence + fused fc ----
            with tc.tile_pool(name="rec_sbuf", bufs=3) as rp, \
                 tc.tile_pool(name="rec_psum", bufs=2, space="PSUM") as rpp, \
                 tc.tile_pool(name="fc_psum", bufs=2, space="PSUM") as fpp:

                psf, maskch = {}, None
                for t in range(0 if "rec" in SKIP else T):
                    sl_in = (t % 2) * 64
                    sl_out = ((t + 1) % 2) * 64
                    if t % R == 0:
                        maskch = rp.tile([P, R * 64], U8, tag="maskch")
                        nc.sync.dma_start(
                            maskch[:], AP(dt["mask_b"], t * P * 64,
                                          [[64, P], [P * 64, R], [1, 64]]))
                    mk = maskch[:, (t % R) * 64:(t % R) * 64 + 64]

                    # gates: per m-window one contiguous accumulation group
                    # (a start=True poisons the whole 2KB PSUM bank for any
                    # other window's pending accumulation, so no interleave)
                    ps_cur = {}
                    for d in dirs:
                        ps = rpp.tile([P, 256], F32, name=f"ps{d}", tag=f"ps{d}")
                        ps_cur[d] = ps
                        tt = t if d == "f" else T - 1 - t
                        for m in range(16):
                            sl = ps[:, m * B:(m + 1) * B]
                            nc.tensor.matmul(sl, b16[d][:],
                                             oh16[:, m * B:(m + 1) * B],
                                             start=True, stop=False)
                            for k in range(2):
                                nc.tensor.matmul(
                                    sl, wih[d][:, (k * 16 + m) * P:(k * 16 + m + 1) * P],
                                    embT[:, k * T * B + tt * B: k * T * B + tt * B + B],
                                    start=False, stop=False)
                            for k in range(4):
                                nc.tensor.matmul(
                                    sl, whh[d][:, (k * 16 + m) * P:(k * 16 + m + 1) * P],
                                    hcur[d][:, sl_in + k * B: sl_in + (k + 1) * B],
                                    start=False, stop=(k == 3))

                    sg = {}
                    ps_dbg = dict(ps_cur)
                    for d in dirs:
                        sg[d] = rp.tile([P, 256], F32, name=f"sg{d}", tag=f"sg{d}")
                        nc.scalar.activation(sg[d][:], ps_cur[d][:], SIG)

                    # f-chain first on DVE, then b-chain, masking tail last,
                    # to avoid head-of-line blocking across the two chains.
                    th, cfb = {}, {}
                    for d in dirs:
                        cfb[d] = rp.tile([P, 64], F32, name=f"cfb{d}", tag=f"cf{d}")
                        nc.gpsimd.tensor_tensor(cfb[d][:], sg[d][:, 64:128],
                                                cst[d][:], op=MUL)
                    t2, cn = {}, None
                    for d in dirs:
                        # gm1 = 1 - 2*sig = tanh of original g (g rows scaled -2)
                        nc.vector.tensor_scalar(
                            out=sg[d][:, 192:256], in0=sg[d][:, 192:256],
                            scalar1=-2.0, scalar2=1.0, op0=MUL, op1=ADD)
                        t2[d] = rp.tile([P, 64], F32, name=f"t2{d}", tag=f"t2{d}")
                        nc.vector.tensor_tensor(t2[d][:], sg[d][:, 0:64],
                                                sg[d][:, 192:256], op=MUL)
                        if d == "f":
                            nc.vector.tensor_tensor(cst[d][:], cfb[d][:], t2[d][:], op=ADD)
                        else:
                            cn = rp.tile([P, 64], F32, tag="cn")
                            nc.vector.tensor_tensor(cn[:], cfb[d][:], t2[d][:], op=ADD)
                            nc.vector.copy_predicated(cst[d][:], mk, cn[:])
                        th[d] = rp.tile([P, 64], F32, name=f"th{d}", tag=f"th{d}")
                        nc.scalar.activation(th[d][:], cst[d][:], TANH)
                    hslot = {d: hcur[d][:, sl_out:sl_out + 64] for d in dirs}
                    nc.vector.tensor_copy(hslot["b"], hcur["b"][:, sl_in:sl_in + 64])
                    nc.vector.tensor_tensor(hslot["f"], sg["f"][:, 128:192],
                                            th["f"][:], op=MUL)
                    hn = rp.tile([P, 64], BF16, tag="hn")
                    nc.vector.tensor_tensor(hn[:], sg["b"][:, 128:192],
                                            th["b"][:], op=MUL)
                    nc.vector.copy_predicated(hslot["b"], mk, hn[:])

                    if DBG0 and t == 0:
                        for d in dirs:
                            psc_ = rp.tile([P, 256], F32, name="psc_", tag="psc_")
                            nc.vector.tensor_copy(psc_[:], ps_dbg[d][:])
                            nc.sync.dma_start(dps_o[d].ap()[:], psc_[:])
                            nc.sync.dma_start(dsg_o[d].ap()[:], sg[d][:])
                            nc.sync.dma_start(dc_o[d].ap()[:], cst[d][:])
                            nc.sync.dma_start(
                                dh_o[d].ap()[:],
                                hcur[d][:, sl_out:sl_out + 64])

                    # fused fc: accumulate emissions^T for this step
                    for d in dirs:
                        if t % 32 == 0:
                            psf[d] = fpp.tile([K, 512], F32, name=f"psf{d}", tag=f"psf{d}")
                        cc = t % 32 if d == "f" else 31 - (t % 32)
                        for k in range(4):
                            nc.tensor.matmul(
                                psf[d][:, cc * B:(cc + 1) * B],
                                fcw[d][:, k * K:(k + 1) * K],
                                hcur[d][:, sl_out + k * B: sl_out + (k + 1) * B],
                                start=(k == 0), stop=(k == 3))
                    if t % 32 == 31:
                        cf_, cb_ = t // 32, NCK - 1 - t // 32
                        for d, ck in (("f", cf_), ("b", cb_)):
                            esl = emisT[:, ck * 512:(ck + 1) * 512]
                            first = (ck < NCK // 2) == (d == "f")
                            if first:
                                nc.vector.tensor_copy(esl, psf[d][:])
                            else:
                                nc.vector.tensor_tensor(esl, esl, psf[d][:], op=ADD)

            nc.sync.dma_start(demis_o.ap()[:], emisT[:])

            # ---- CRF forward (exp domain) ----
            with tc.tile_pool(name="crf_sbuf", bufs=2) as cp, \
                 tc.tile_pool(name="crf_persist", bufs=1) as cpr, \
                 tc.tile_pool(name="crf_psum", bufs=2, space="PSUM") as cpp:
                transTs = cpr.tile([K, K], F32, tag="transTs")
                nc.sync.dma_start(transTs[:], dt["transT"].ap()[:])
                ET = cpr.tile([K, K], F32, tag="ET")
                nc.scalar.activation(ET[:], transTs[:], mybir.ActivationFunctionType.Exp)
                Estop = cpr.tile([K, 1], F32, tag="Estop")
                nc.scalar.activation(Estop[:], transTs[:, STOP:STOP + 1],
                                     mybir.ActivationFunctionType.Exp)
                ones12 = cpr.tile([K, K], F32, tag="ones12")
                nc.vector.memset(ones12[:], 1.0)
                fcb_p = cpr.tile([K, 1], F32, tag="fcb_p")
                nc.sync.dma_start(fcb_p[:], AP(dt["fcb"], 0, [[1, K], [1, 1]]))
                expem = cpr.tile([K, T * B], F32, tag="expem")
                nc.scalar.activation(expem[:], emisT[:],
                                     mybir.ActivationFunctionType.Exp, bias=fcb_p[:, 0:1])
                a0 = cpr.tile([K, B], F32, tag="a0")
                nc.sync.dma_start(a0[:], dt["a0"].ap()[:])
                hist = cpr.tile([K, T * B], F32, tag="hist")
                Lh = cpr.tile([1, NE * B], F32, tag="Lh")
                nc.vector.memset(Lh[:], 0.0)

                rhs = a0
                rhs_sl = (0, B)
                for t in range(0 if "crf" in SKIP else T):
                    psc = cpp.tile([K, B], F32, tag="psc")
                    nc.tensor.matmul(psc[:], ET[:],
                                     rhs[:, rhs_sl[0]:rhs_sl[1]],
                                     start=True, stop=True)
                    nc.vector.tensor_tensor(hist[:, t * B:(t + 1) * B], psc[:],
                                            expem[:, t * B:(t + 1) * B], op=MUL)
                    rhs, rhs_sl = hist, (t * B, (t + 1) * B)
                    if t % R == R - 1 and t < T - 1:
                        j = (t + 1) // R
                        pss = cpp.tile([K, B], F32, tag="pss", bufs=1)
                        nc.tensor.matmul(pss[:], ones12[:], hist[:, t * B:(t + 1) * B],
                                         start=True, stop=True)
                        Ssb = cp.tile([K, B], F32, tag="Ssb")
                        nc.vector.tensor_copy(Ssb[:], pss[:])
                        rS = cp.tile([K, B], F32, tag="rS")
                        nc.vector.reciprocal(rS[:], Ssb[:])
                        rn = cp.tile([K, B], F32, tag="rn")
                        nc.vector.tensor_tensor(rn[:], hist[:, t * B:(t + 1) * B],
                                                rS[:], op=MUL)
                        lnS = cp.tile([1, B], F32, tag="lnS")
                        nc.scalar.activation(lnS[:], Ssb[0:1, :],
                                             mybir.ActivationFunctionType.Ln)
                        nc.vector.tensor_tensor(Lh[:, j * B:(j + 1) * B],
                                                Lh[:, (j - 1) * B:j * B], lnS[:], op=ADD)
                        rhs, rhs_sl = rn, (0, B)

                # capture at t = len-1
                mselb = cpr.tile([K, T * B], F32, tag="mselb")
                nc.sync.dma_start(mselb[:], dt["msel"].ap()[:])
                nc.vector.tensor_tensor(hist[:], hist[:], mselb[:], op=MUL)
                aend = cp.tile([K, B], F32, tag="aend")
                nc.vector.tensor_reduce(aend[:], fv(hist, 0, [[1, B], [B, T]]),
                                        axis=X, op=ADD)
                mep = cp.tile([1, NE * B], F32, tag="mep")
                nc.sync.dma_start(mep[:], AP(dt["maskep"], 0, [[1, 1], [1, NE * B]]))
                prod5 = cp.tile([1, NE * B], F32, tag="prod5")
                nc.vector.tensor_tensor(prod5[:], Lh[:], mep[:], op=MUL)
                Lend = cp.tile([1, B], F32, tag="Lend")
                nc.vector.tensor_reduce(Lend[:], fv(prod5, 0, [[1, B], [B, NE]]),
                                        axis=X, op=ADD)
                azs = cp.tile([K, B], F32, tag="azs")
                nc.vector.tensor_scalar(out=azs[:], in0=aend[:], scalar1=Estop[:, 0:1],
                                        scalar2=None, op0=MUL)
                ps2 = cpp.tile([K, B], F32, tag="ps2", bufs=1)
                nc.tensor.matmul(ps2[:], ones12[:], azs[:], start=True, stop=True)
                logz0 = cp.tile([1, B], F32, tag="logz0")
                nc.scalar.activation(logz0[:], ps2[0:1, :],
                                     mybir.ActivationFunctionType.Ln)
                logzf = cp.tile([1, B], F32, tag="logzf")
                nc.vector.tensor_tensor(logzf[:], logz0[:], Lend[:], op=ADD)
                nc.sync.dma_start(AP(dlogz_o, 0, [[1, 1], [1, B]]), logzf[:])

                # ---- gold score ----
                tfl = cp.tile([1, 144], F32, tag="tfl")
                nc.sync.dma_start(tfl[:], AP(dt["trans"], 0, [[1, 1], [1, 144]]))
                tfb = cp.tile([B, 144], F32, tag="tfb")
                nc.gpsimd.partition_broadcast(tfb[:], tfl[:])
                cnts = cp.tile([B, 144], F32, tag="cnts")
                nc.sync.dma_start(cnts[:], dt["counts"].ap()[:])
                pr1 = cp.tile([B, 144], F32, tag="pr1")
                nc.vector.tensor_tensor(pr1[:], cnts[:], tfb[:], op=MUL)
                g1 = cp.tile([B, 1], F32, tag="g1")
                nc.vector.tensor_reduce(g1[:], pr1[:], axis=X, op=ADD)
                fcbr = cp.tile([1, K], F32, tag="fcbr")
                nc.sync.dma_start(fcbr[:], AP(dt["fcb"], 0, [[1, 1], [1, K]]))
                fcbb = cp.tile([B, K], F32, tag="fcbb")
                nc.gpsimd.partition_broadcast(fcbb[:], fcbr[:])
                cntbs = cp.tile([B, K], F32, tag="cntbs")
                nc.sync.dma_start(cntbs[:], dt["cntb"].ap()[:])
                pr2 = cp.tile([B, K], F32, tag="pr2")
                nc.vector.tensor_tensor(pr2[:], cntbs[:], fcbb[:], op=MUL)
                g2 = cp.tile([B, 1], F32, tag="g2")
                nc.vector.tensor_reduce(g2[:], pr2[:], axis=X, op=ADD)
                g12 = cp.tile([B, 1], F32, tag="g12")
                nc.vector.tensor_tensor(g12[:], g1[:], g2[:], op=ADD)
                nc.sync.dma_start(AP(scr16, 0, [[1, B], [1, 1]]), g12[:])
                g12r = cp.tile([1, B], F32, tag="g12r")
                nc.sync.dma_start(g12r[:], AP(scr16, 0, [[1, 1], [1, B]]))

                selb = cpr.tile([K, T * B], F32, tag="selb")
                nc.sync.dma_start(selb[:], dt["sel"].ap()[:])
                nc.vector.tensor_tensor(selb[:], emisT[:], selb[:], op=MUL)
                g3 = cp.tile([K, B], F32, tag="g3")
                nc.vector.tensor_reduce(g3[:], fv(selb, 0, [[1, B], [B, T]]),
                                        axis=X, op=ADD)
                ps3 = cpp.tile([K, B], F32, tag="ps3", bufs=1)
                nc.tensor.matmul(ps3[:], ones12[:], g3[:], start=True, stop=True)
                goldT = cp.tile([1, B], F32, tag="goldT")
                nc.vector.tensor_tensor(goldT[:], g12r[:], ps3[0:1, :], op=ADD)
                nc.sync.dma_start(AP(dgold_o, 0, [[1, 1], [1, B]]), goldT[:])
                nllT = cp.tile([1, B], F32, tag="nllT")
                nc.vector.tensor_tensor(nllT[:], logzf[:], goldT[:], op=SUB)
                nc.sync.dma_start(AP(nll_o, 0, [[1, 1], [1, B]]), nllT[:])
    return nc


_CACHE = {}


def get_program():
    if "nc" not in _CACHE:
        nc = bacc.Bacc("TRN2", target_bir_lowering=False, debug=False,
                       num_devices=NCORES)
        build(nc)
        nc.compile()
        _CACHE["nc"] = nc
    return _CACHE["nc"]


def perm_ifog(w):
    # [4H, ...] rows i,f,g,o -> i,f,o,g
    return np.concatenate([w[0:512], w[512:1024], w[1536:2048], w[1024:1536]], 0)


def host_prep(inputs):
    f32 = np.float32
    bf = ml_dtypes.bfloat16
    x = np.asarray(inputs["x"]).astype(np.int64)
    lengths = np.asarray(inputs["lengths"]).astype(np.int64)
    tags = np.asarray(inputs["tags"]).astype(np.int64)
    emb = np.asarray(inputs["embedding"], f32)
    trans = np.asarray(inputs["trans"], f32)
    fcW = np.asarray(inputs["fc_W"], f32)
    fcb = np.asarray(inputs["fc_b"], f32)
    h0 = np.asarray(inputs["h0"], f32)
    c0 = np.asarray(inputs["c0"], f32)

    Wd, Bd = {}, {}
    for d in ("f", "b"):
        wih = perm_ifog(np.asarray(inputs[f"W_ih_{d}"], f32)).copy()
        whh = perm_ifog(np.asarray(inputs[f"W_hh_{d}"], f32)).copy()
        bi = perm_ifog(np.asarray(inputs[f"b_ih_{d}"], f32)[:, None])[:, 0]
        bh = perm_ifog(np.asarray(inputs[f"b_hh_{d}"], f32)[:, None])[:, 0]
        bsum = (bi + bh).copy()
        # scale g rows by -2: tanh(g) = 1 - 2*sigmoid(-2g)
        wih[1536:2048] *= -2.0
        whh[1536:2048] *= -2.0
        bsum[1536:2048] *= -2.0
        Wd[d] = (wih.T.astype(bf).copy(), whh.T.astype(bf).copy())
        Bd[d] = bsum.reshape(16, P).astype(bf).copy()

    fcWT = {"f": fcW[:, :H].T.astype(bf).copy(), "b": fcW[:, H:].T.astype(bf).copy()}
    oh16 = np.zeros((16, 256), f32)
    for r in range(16):
        oh16[r, r * B:(r + 1) * B] = 1.0
    oh16 = oh16.astype(bf)

    maps = []
    for c in range(NCORES):
        bs = slice(c * B, (c + 1) * B)
        xs = x[bs]            # [16, T]
        ln = lengths[bs]      # [16]
        tg = tags[bs]         # [16, T]
        m = {"trans": trans, "transT": trans.T.astype(f32).copy(), "fcb": fcb,
             "onehot16": oh16}
        # host embedding gather (pure indexing): embT[p, k*T*B + t*16 + b]
        xe = emb[xs]                                  # [16, T, E]
        m["xembT"] = np.ascontiguousarray(
            xe.transpose(2, 1, 0).reshape(2, P, T * B)
        ).reshape(E, T * B).astype(bf)
        for d in ("f", "b"):
            m[f"wihT_{d}"], m[f"whhT_{d}"] = Wd[d]
            m[f"bias16_{d}"] = Bd[d]
            m[f"fcWT_{d}"] = fcWT[d]
            di = 0 if d == "f" else 1
            h0T = h0[di, bs].T.reshape(4, P, B).transpose(1, 0, 2).reshape(P, 64)
            c0T = c0[di, bs].T.reshape(4, P, B).transpose(1, 0, 2).reshape(P, 64)
            m[f"h0T_{d}"] = h0T.astype(bf).copy()
            m[f"c0T_{d}"] = c0T.astype(f32).copy()
        # bwd mask: step s processes tau = T-1-s; valid iff tau < len
        tau = (T - 1 - np.arange(T))[:, None]          # [T, 1]
        mk = (tau < ln[None, :]).astype(f32)           # [T, 16]
        m["mask_b"] = np.broadcast_to(
            mk[:, None, None, :], (T, P, 4, B)).reshape(T, P, 64).astype(np.uint8).copy()
        a0 = np.zeros((K, B), f32); a0[START, :] = 1.0
        m["a0"] = a0
        msel = np.zeros((K, T, B), f32)
        msel[:, ln - 1, np.arange(B)] = 1.0
        m["msel"] = msel.reshape(K, T * B)
        mep = np.zeros((NE, B), f32)
        mep[(ln - 1) // R, np.arange(B)] = 1.0
        m["maskep"] = mep.reshape(-1)
        tarange = np.arange(T)[None, :]
        valid = tarange < ln[:, None]                  # [16, T]
        selm = np.zeros((K, T, B), f32)
        jj = np.arange(K)[:, None, None]
        selm[:] = (tg.T[None] == jj) & valid.T[None]
        m["sel"] = np.ascontiguousarray(selm.reshape(K, T * B))
        counts = np.zeros((B, 144), f32)
        cntb = np.zeros((B, K), f32)
        for b in range(B):
            L = int(ln[b])
            prev = START
            for t in range(L):
                nx = int(tg[b, t])
                counts[b, nx * K + prev] += 1
                cntb[b, nx] += 1
                prev = nx
            counts[b, STOP * K + prev] += 1
        m["counts"] = counts
        m["cntb"] = cntb
        maps.append(m)
    return maps


def kernel(**inputs):
    from concourse.bass_utils import run_bass_kernel_spmd
    nc = get_program()
    maps = host_prep(inputs)
    res = run_bass_kernel_spmd(nc, maps, core_ids=list(range(NCORES)))
    out = np.concatenate([r["nll"] for r in res.results]).astype(np.float32)
    kernel.last_results = res
    return out


# revision 14
# speedup vs baseline: 3.7193x; 3.7193x over previous
"""BiLSTM-CRF NLL kernel for 8 TRN2 NeuronCores.

Sharding: data-parallel over batch. B=128 split into 8 shards of 16
sentences; each core runs both LSTM directions, the fc projection (fused
per-step), the CRF forward pass (exp-domain, renorm every R=8 steps,
capture-at-length), and the gold-path score for its shard.

Recurrence design (per core, per step t, per direction d):
  gates^T [4H=2048, B=16] live in one PSUM tile ps [128, 256], col = m*16+b,
  gate row order permuted to [i | f | o | g]; g rows of W/b pre-scaled by -2
  so tanh(g) = 1 - 2*sigmoid(-2g) comes out of the same sigmoid op.
  Accumulation per m-window: bias (one-hot rhs matmul, start=True) then
  2 k-tiles of Wih @ emb(x_t) (embT resident in SBUF, host-gathered) then
  4 k-tiles of Whh @ h (stop=True).
  Elementwise: 1 sigmoid (Act, 256w) -> gm1 (DVE) -> t2 (DVE), cf (Pool),
  c (DVE, predicated for bwd), tanh_c (Act), h (DVE, bf16 -> hcur slot).
  fc fused: per-step matmuls accumulate emissions^T [12, 512] per 32-step
  chunk in PSUM; first-finishing direction copies to emisT, other adds.
CRF: alpha'_{t+1} = (E @ alpha'_t) * exp(emis_t + fc_b), E = exp(trans)^T;
  renorm every R=8; alpha history in SBUF; per-sentence capture at len-1.
"""

import os
import numpy as np
import ml_dtypes

import concourse.bass as bass
import concourse.bacc as bacc
import concourse.mybir as mybir
import concourse.tile as tile
from concourse.bass import AP

F32 = mybir.dt.float32
BF16 = mybir.dt.bfloat16
I32 = mybir.dt.int32
U8 = mybir.dt.uint8
MUL = mybir.AluOpType.mult
ADD = mybir.AluOpType.add
SUB = mybir.AluOpType.subtract
X = mybir.AxisListType.X
SIG = mybir.ActivationFunctionType.Sigmoid
TANH = mybir.ActivationFunctionType.Tanh

P = 128
B = 16            # batch per core
H = 512
E = 256
G = 2048          # 4H
K = 12
START, STOP = 10, 11
R = 8             # CRF renorm period
NCORES = 8

T = int(os.environ.get("BASS_LSTM_T", "256"))
SKIP = set(os.environ.get("BASS_SKIP", "").split(","))
NE = T // R
NCK = T // 32     # fc chunks


def fv(t, off, pat):
    """Free-dim view of a contiguous [P, F] tile: keep partition pair, replace
    free dims with `pat` (list of [step, count]) at element offset `off`."""
    base = t[:] if not isinstance(t, AP) else t
    part = list(base.ap[0])
    return AP(base.tensor, base.offset + off, [part] + [list(p) for p in pat])


def build(nc):
    dirs = ("f", "b")
    dt = {}

    def din(name, shape, dtype):
        dt[name] = nc.dram_tensor(name, shape, dtype, kind="ExternalInput")
        return dt[name]

    for d in dirs:
        din(f"wihT_{d}", [E, G], BF16)
        din(f"whhT_{d}", [H, G], BF16)
        din(f"bias16_{d}", [16, P], BF16)
        din(f"h0T_{d}", [P, 64], BF16)
        din(f"c0T_{d}", [P, 64], F32)
        din(f"fcWT_{d}", [H, K], BF16)
    din("xembT", [E, T * B], BF16)
    din("onehot16", [16, 256], BF16)
    din("mask_b", [T, P, 64], U8)
    din("transT", [K, K], F32)
    din("trans", [K, K], F32)
    din("fcb", [K], F32)
    din("a0", [K, B], F32)
    din("msel", [K, T * B], F32)
    din("maskep", [NE * B], F32)
    din("sel", [K, T * B], F32)
    din("counts", [B, 144], F32)
    din("cntb", [B, K], F32)

    nll_o = nc.dram_tensor("nll", [B], F32, kind="ExternalOutput")
    demis_o = nc.dram_tensor("dbg_emis", [K, T * B], F32, kind="ExternalOutput")
    dlogz_o = nc.dram_tensor("dbg_logz", [B], F32, kind="ExternalOutput")
    dgold_o = nc.dram_tensor("dbg_gold", [B], F32, kind="ExternalOutput")

    scr16 = nc.dram_tensor("scr16", [B], F32)
    DBG0 = os.environ.get("BASS_DBG0") == "1"
    if DBG0:
        dsg_o = {d: nc.dram_tensor(f"dbg_sg_{d}", [P, 256], F32,
                                   kind="ExternalOutput") for d in dirs}
        dh_o = {d: nc.dram_tensor(f"dbg_h_{d}", [P, 64], BF16,
                                  kind="ExternalOutput") for d in dirs}
        dc_o = {d: nc.dram_tensor(f"dbg_c_{d}", [P, 64], F32,
                                  kind="ExternalOutput") for d in dirs}
        dps_o = {d: nc.dram_tensor(f"dbg_ps_{d}", [P, 256], F32,
                                   kind="ExternalOutput") for d in dirs}

    with tile.TileContext(nc) as tc:
        with tc.tile_pool(name="persist", bufs=1) as pp:
            whh = {d: pp.tile([P, 4 * 16 * P], BF16, name=f"whh{d}", tag=f"whh{d}") for d in dirs}
            wih = {d: pp.tile([P, 2 * 16 * P], BF16, name=f"wih{d}", tag=f"wih{d}") for d in dirs}
            b16 = {d: pp.tile([16, P], BF16, name=f"b16{d}", tag=f"b16{d}") for d in dirs}
            fcw = {d: pp.tile([P, 4 * K], BF16, name=f"fcw{d}", tag=f"fcw{d}") for d in dirs}
            hcur = {d: pp.tile([P, 2 * 64], BF16, name=f"hcur{d}", tag=f"hcur{d}") for d in dirs}
            cst = {d: pp.tile([P, 64], F32, name=f"cst{d}", tag=f"c{d}") for d in dirs}
            oh16 = pp.tile([16, 256], BF16, tag="oh16")
            embT = pp.tile([P, 2 * T * B], BF16, tag="embT")
            emisT = pp.tile([K, T * B], F32, tag="emisT")

            for d in dirs:
                for k in range(4):
                    nc.sync.dma_start(
                        whh[d][:, k * 16 * P:(k + 1) * 16 * P],
                        dt[f"whhT_{d}"].ap()[k * P:(k + 1) * P, :])
                    nc.sync.dma_start(
                        fcw[d][:, k * K:(k + 1) * K],
                        dt[f"fcWT_{d}"].ap()[k * P:(k + 1) * P, :])
                for k in range(2):
                    nc.sync.dma_start(
                        wih[d][:, k * 16 * P:(k + 1) * 16 * P],
                        dt[f"wihT_{d}"].ap()[k * P:(k + 1) * P, :])
                nc.sync.dma_start(b16[d][:], dt[f"bias16_{d}"].ap()[:])
                nc.sync.dma_start(hcur[d][:, 0:64], dt[f"h0T_{d}"].ap()[:])
                nc.sync.dma_start(cst[d][:], dt[f"c0T_{d}"].ap()[:])
            nc.sync.dma_start(oh16[:], dt["onehot16"].ap()[:])
            for k in range(2):
                nc.sync.dma_start(embT[:, k * T * B:(k + 1) * T * B],
                                  dt["xembT"].ap()[k * P:(k + 1) * P, :])

            # ---- recurrence + fused fc ----
            with tc.tile_pool(name="rec_sbuf", bufs=3) as rp, \
                 tc.tile_pool(name="rec_psum", bufs=2, space="PSUM") as rpp, \
                 tc.tile_pool(name="fc_psum", bufs=2, space="PSUM") as fpp:

                psf, maskch = {}, None
                for t in range(0 if "rec" in SKIP else T):
                    sl_in = (t % 2) * 64
                    sl_out = ((t + 1) % 2) * 64
                    if t % R == 0:
                        maskch = rp.tile([P, R * 64], U8, tag="maskch")
                        nc.sync.dma_start(
                            maskch[:], AP(dt["mask_b"], t * P * 64,
                                          [[64, P], [P * 64, R], [1, 64]]))
                    mk = maskch[:, (t % R) * 64:(t % R) * 64 + 64]

                    # gates: per m-window one contiguous accumulation group
                    # (a start=True poisons the whole 2KB PSUM bank for any
                    # other window's pending accumulation, so no interleave)
                    ps_cur = {}
                    for d in dirs:
                        ps = rpp.tile([P, 256], F32, name=f"ps{d}", tag=f"ps{d}")
                        ps_cur[d] = ps
                        tt = t if d == "f" else T - 1 - t
                        for m in range(16):
                            sl = ps[:, m * B:(m + 1) * B]
                            nc.tensor.matmul(sl, b16[d][:],
                                             oh16[:, m * B:(m + 1) * B],
                                             start=True, stop=False)
                            for k in range(2):
                                nc.tensor.matmul(
                                    sl, wih[d][:, (k * 16 + m) * P:(k * 16 + m + 1) * P],
                                    embT[:, k * T * B + tt * B: k * T * B + tt * B + B],
                                    start=False, stop=False)
                            for k in range(4):
                                nc.tensor.matmul(
                                    sl, whh[d][:, (k * 16 + m) * P:(k * 16 + m + 1) * P],
                                    hcur[d][:, sl_in + k * B: sl_in + (k + 1) * B],
                                    start=False, stop=(k == 3))

                    sg = {}
                    ps_dbg = dict(ps_cur)
                    for d in dirs:
                        sg[d] = rp.tile([P, 256], F32, name=f"sg{d}", tag=f"sg{d}")
                        nc.scalar.activation(sg[d][:, 0:192], ps_cur[d][:, 0:192], SIG)
                    for d in dirs:
                        nc.scalar.activation(sg[d][:, 192:256], ps_cur[d][:, 192:256], SIG)

                    # f-chain first on DVE, then b-chain, masking tail last,
                    # to avoid head-of-line blocking across the two chains.
                    th, cfb = {}, {}
                    for d in dirs:
                        cfb[d] = rp.tile([P, 64], F32, name=f"cfb{d}", tag=f"cf{d}")
                        nc.gpsimd.tensor_tensor(cfb[d][:], sg[d][:, 128:192],
                                                cst[d][:], op=MUL)
                    t2, cn = {}, None
                    for d in dirs:
                        # gm1 = 1 - 2*sig = tanh of original g (g rows scaled -2)
                        nc.vector.tensor_scalar(
                            out=sg[d][:, 0:64], in0=sg[d][:, 0:64],
                            scalar1=-2.0, scalar2=1.0, op0=MUL, op1=ADD)
                        t2[d] = rp.tile([P, 64], F32, name=f"t2{d}", tag=f"t2{d}")
                        nc.vector.tensor_tensor(t2[d][:], sg[d][:, 64:128],
                                                sg[d][:, 0:64], op=MUL)
                        if d == "f":
                            nc.vector.tensor_tensor(cst[d][:], cfb[d][:], t2[d][:], op=ADD)
                        else:
                            cn = rp.tile([P, 64], F32, tag="cn")
                            nc.vector.tensor_tensor(cn[:], cfb[d][:], t2[d][:], op=ADD)
                            nc.vector.copy_predicated(cst[d][:], mk, cn[:])
                        th[d] = rp.tile([P, 64], F32, name=f"th{d}", tag=f"th{d}")
                        nc.scalar.activation(th[d][:], cst[d][:], TANH)
                    hslot = {d: hcur[d][:, sl_out:sl_out + 64] for d in dirs}
                    nc.vector.tensor_copy(hslot["b"], hcur["b"][:, sl_in:sl_in + 64])
                    nc.vector.tensor_tensor(hslot["f"], sg["f"][:, 192:256],
                                            th["f"][:], op=MUL)
                    hn = rp.tile([P, 64], BF16, tag="hn")
                    nc.vector.tensor_tensor(hn[:], sg["b"][:, 192:256],
                                            th["b"][:], op=MUL)
                    nc.vector.copy_predicated(hslot["b"], mk, hn[:])

                    if DBG0 and t == 0:
                        for d in dirs:
                            psc_ = rp.tile([P, 256], F32, name="psc_", tag="psc_")
                            nc.vector.tensor_copy(psc_[:], ps_dbg[d][:])
                            nc.sync.dma_start(dps_o[d].ap()[:], psc_[:])
                            nc.sync.dma_start(dsg_o[d].ap()[:], sg[d][:])
                            nc.sync.dma_start(dc_o[d].ap()[:], cst[d][:])
                            nc.sync.dma_start(
                                dh_o[d].ap()[:],
                                hcur[d][:, sl_out:sl_out + 64])

                    # fused fc: accumulate emissions^T for this step
                    for d in dirs:
                        if t % 32 == 0:
                            psf[d] = fpp.tile([K, 512], F32, name=f"psf{d}", tag=f"psf{d}")
                        cc = t % 32 if d == "f" else 31 - (t % 32)
                        for k in range(4):
                            nc.tensor.matmul(
                                psf[d][:, cc * B:(cc + 1) * B],
                                fcw[d][:, k * K:(k + 1) * K],
                                hcur[d][:, sl_out + k * B: sl_out + (k + 1) * B],
                                start=(k == 0), stop=(k == 3))
                    if t % 32 == 31:
                        cf_, cb_ = t // 32, NCK - 1 - t // 32
                        for d, ck in (("f", cf_), ("b", cb_)):
                            esl = emisT[:, ck * 512:(ck + 1) * 512]
                            first = (ck < NCK // 2) == (d == "f")
                            if first:
                                nc.vector.tensor_copy(esl, psf[d][:])
                            else:
                                nc.vector.tensor_tensor(esl, esl, psf[d][:], op=ADD)

            nc.sync.dma_start(demis_o.ap()[:], emisT[:])

            # ---- CRF forward (exp domain) ----
            with tc.tile_pool(name="crf_sbuf", bufs=2) as cp, \
                 tc.tile_pool(name="crf_persist", bufs=1) as cpr, \
                 tc.tile_pool(name="crf_psum", bufs=2, space="PSUM") as cpp:
                transTs = cpr.tile([K, K], F32, tag="transTs")
                nc.sync.dma_start(transTs[:], dt["transT"].ap()[:])
                ET = cpr.tile([K, K], F32, tag="ET")
                nc.scalar.activation(ET[:], transTs[:], mybir.ActivationFunctionType.Exp)
                Estop = cpr.tile([K, 1], F32, tag="Estop")
                nc.scalar.activation(Estop[:], transTs[:, STOP:STOP + 1],
                                     mybir.ActivationFunctionType.Exp)
                ones12 = cpr.tile([K, K], F32, tag="ones12")
                nc.vector.memset(ones12[:], 1.0)
                fcb_p = cpr.tile([K, 1], F32, tag="fcb_p")
                nc.sync.dma_start(fcb_p[:], AP(dt["fcb"], 0, [[1, K], [1, 1]]))
                expem = cpr.tile([K, T * B], F32, tag="expem")
                nc.scalar.activation(expem[:], emisT[:],
                                     mybir.ActivationFunctionType.Exp, bias=fcb_p[:, 0:1])
                a0 = cpr.tile([K, B], F32, tag="a0")
                nc.sync.dma_start(a0[:], dt["a0"].ap()[:])
                hist = cpr.tile([K, T * B], F32, tag="hist")
                Lh = cpr.tile([1, NE * B], F32, tag="Lh")
                nc.vector.memset(Lh[:], 0.0)

                rhs = a0
                rhs_sl = (0, B)
                for t in range(0 if "crf" in SKIP else T):
                    psc = cpp.tile([K, B], F32, tag="psc")
                    nc.tensor.matmul(psc[:], ET[:],
                                     rhs[:, rhs_sl[0]:rhs_sl[1]],
                                     start=True, stop=True)
                    nc.vector.tensor_tensor(hist[:, t * B:(t + 1) * B], psc[:],
                                            expem[:, t * B:(t + 1) * B], op=MUL)
                    rhs, rhs_sl = hist, (t * B, (t + 1) * B)
                    if t % R == R - 1 and t < T - 1:
                        j = (t + 1) // R
                        pss = cpp.tile([K, B], F32, tag="pss", bufs=1)
                        nc.tensor.matmul(pss[:], ones12[:], hist[:, t * B:(t + 1) * B],
                                         start=True, stop=True)
                        Ssb = cp.tile([K, B], F32, tag="Ssb")
                        nc.vector.tensor_copy(Ssb[:], pss[:])
                        rS = cp.tile([K, B], F32, tag="rS")
                        nc.vector.reciprocal(rS[:], Ssb[:])
                        rn = cp.tile([K, B], F32, tag="rn")
                        nc.vector.tensor_tensor(rn[:], hist[:, t * B:(t + 1) * B],
                                                rS[:], op=MUL)
                        lnS = cp.tile([1, B], F32, tag="lnS")
                        nc.scalar.activation(lnS[:], Ssb[0:1, :],
                                             mybir.ActivationFunctionType.Ln)
                        nc.vector.tensor_tensor(Lh[:, j * B:(j + 1) * B],
                                                Lh[:, (j - 1) * B:j * B], lnS[:], op=ADD)
                        rhs, rhs_sl = rn, (0, B)

                # capture at t = len-1
                mselb = cpr.tile([K, T * B], F32, tag="mselb")
                nc.sync.dma_start(mselb[:], dt["msel"].ap()[:])
                nc.vector.tensor_tensor(hist[:], hist[:], mselb[:], op=MUL)
                aend = cp.tile([K, B], F32, tag="aend")
                nc.vector.tensor_reduce(aend[:], fv(hist, 0, [[1, B], [B, T]]),
                                        axis=X, op=ADD)
                mep = cp.tile([1, NE * B], F32, tag="mep")
                nc.sync.dma_start(mep[:], AP(dt["maskep"], 0, [[1, 1], [1, NE * B]]))
                prod5 = cp.tile([1, NE * B], F32, tag="prod5")
                nc.vector.tensor_tensor(prod5[:], Lh[:], mep[:], op=MUL)
                Lend = cp.tile([1, B], F32, tag="Lend")
                nc.vector.tensor_reduce(Lend[:], fv(prod5, 0, [[1, B], [B, NE]]),
                                        axis=X, op=ADD)
                azs = cp.tile([K, B], F32, tag="azs")
                nc.vector.tensor_scalar(out=azs[:], in0=aend[:], scalar1=Estop[:, 0:1],
                                        scalar2=None, op0=MUL)
                ps2 = cpp.tile([K, B], F32, tag="ps2", bufs=1)
                nc.tensor.matmul(ps2[:], ones12[:], azs[:], start=True, stop=True)
                logz0 = cp.tile([1, B], F32, tag="logz0")
                nc.scalar.activation(logz0[:], ps2[0:1, :],
                                     mybir.ActivationFunctionType.Ln)
                logzf = cp.tile([1, B], F32, tag="logzf")
                nc.vector.tensor_tensor(logzf[:], logz0[:], Lend[:], op=ADD)
                nc.sync.dma_start(AP(dlogz_o, 0, [[1, 1], [1, B]]), logzf[:])

                # ---- gold score ----
                tfl = cp.tile([1, 144], F32, tag="tfl")
                nc.sync.dma_start(tfl[:], AP(dt["trans"], 0, [[1, 1], [1, 144]]))
                tfb = cp.tile([B, 144], F32, tag="tfb")
                nc.gpsimd.partition_broadcast(tfb[:], tfl[:])
                cnts = cp.tile([B, 144], F32, tag="cnts")
                nc.sync.dma_start(cnts[:], dt["counts"].ap()[:])
                pr1 = cp.tile([B, 144], F32, tag="pr1")
                nc.vector.tensor_tensor(pr1[:], cnts[:], tfb[:], op=MUL)
                g1 = cp.tile([B, 1], F32, tag="g1")
                nc.vector.tensor_reduce(g1[:], pr1[:], axis=X, op=ADD)
                fcbr = cp.tile([1, K], F32, tag="fcbr")
                nc.sync.dma_start(fcbr[:], AP(dt["fcb"], 0, [[1, 1], [1, K]]))
                fcbb = cp.tile([B, K], F32, tag="fcbb")
                nc.gpsimd.partition_broadcast(fcbb[:], fcbr[:])
                cntbs = cp.tile([B, K], F32, tag="cntbs")
                nc.sync.dma_start(cntbs[:], dt["cntb"].ap()[:])
                pr2 = cp.tile([B, K], F32, tag="pr2")
                nc.vector.tensor_tensor(pr2[:], cntbs[:], fcbb[:], op=MUL)
                g2 = cp.tile([B, 1], F32, tag="g2")
                nc.vector.tensor_reduce(g2[:], pr2[:], axis=X, op=ADD)
                g12 = cp.tile([B, 1], F32, tag="g12")
                nc.vector.tensor_tensor(g12[:], g1[:], g2[:], op=ADD)
                nc.sync.dma_start(AP(scr16, 0, [[1, B], [1, 1]]), g12[:])
                g12r = cp.tile([1, B], F32, tag="g12r")
                nc.sync.dma_start(g12r[:], AP(scr16, 0, [[1, 1], [1, B]]))

                selb = cpr.tile([K, T * B], F32, tag="selb")
                nc.sync.dma_start(selb[:], dt["sel"].ap()[:])
                nc.vector.tensor_tensor(selb[:], emisT[:], selb[:], op=MUL)
                g3 = cp.tile([K, B], F32, tag="g3")
                nc.vector.tensor_reduce(g3[:], fv(selb, 0, [[1, B], [B, T]]),
                                        axis=X, op=ADD)
                ps3 = cpp.tile([K, B], F32, tag="ps3", bufs=1)
                nc.tensor.matmul(ps3[:], ones12[:], g3[:], start=True, stop=True)
                goldT = cp.tile([1, B], F32, tag="goldT")
                nc.vector.tensor_tensor(goldT[:], g12r[:], ps3[0:1, :], op=ADD)
                nc.sync.dma_start(AP(dgold_o, 0, [[1, 1], [1, B]]), goldT[:])
                nllT = cp.tile([1, B], F32, tag="nllT")
                nc.vector.tensor_tensor(nllT[:], logzf[:], goldT[:], op=SUB)
                nc.sync.dma_start(AP(nll_o, 0, [[1, 1], [1, B]]), nllT[:])
    return nc


_CACHE = {}


def get_program():
    if "nc" not in _CACHE:
        nc = bacc.Bacc("TRN2", target_bir_lowering=False, debug=False,
                       num_devices=NCORES)
        build(nc)
        nc.compile()
        _CACHE["nc"] = nc
    return _CACHE["nc"]


def perm_ifog(w):
    # [4H, ...] rows i,f,g,o -> g,i,f,o (g first so its sigmoid unblocks early)
    return np.concatenate([w[1024:1536], w[0:512], w[512:1024], w[1536:2048]], 0)


def host_prep(inputs):
    f32 = np.float32
    bf = ml_dtypes.bfloat16
    x = np.asarray(inputs["x"]).astype(np.int64)
    lengths = np.asarray(inputs["lengths"]).astype(np.int64)
    tags = np.asarray(inputs["tags"]).astype(np.int64)
    emb = np.asarray(inputs["embedding"], f32)
    trans = np.asarray(inputs["trans"], f32)
    fcW = np.asarray(inputs["fc_W"], f32)
    fcb = np.asarray(inputs["fc_b"], f32)
    h0 = np.asarray(inputs["h0"], f32)
    c0 = np.asarray(inputs["c0"], f32)

    Wd, Bd = {}, {}
    for d in ("f", "b"):
        wih = perm_ifog(np.asarray(inputs[f"W_ih_{d}"], f32)).copy()
        whh = perm_ifog(np.asarray(inputs[f"W_hh_{d}"], f32)).copy()
        bi = perm_ifog(np.asarray(inputs[f"b_ih_{d}"], f32)[:, None])[:, 0]
        bh = perm_ifog(np.asarray(inputs[f"b_hh_{d}"], f32)[:, None])[:, 0]
        bsum = (bi + bh).copy()
        # scale g rows by -2: tanh(g) = 1 - 2*sigmoid(-2g)
        wih[0:512] *= -2.0
        whh[0:512] *= -2.0
        bsum[0:512] *= -2.0
        Wd[d] = (wih.T.astype(bf).copy(), whh.T.astype(bf).copy())
        Bd[d] = bsum.reshape(16, P).astype(bf).copy()

    fcWT = {"f": fcW[:, :H].T.astype(bf).copy(), "b": fcW[:, H:].T.astype(bf).copy()}
    oh16 = np.zeros((16, 256), f32)
    for r in range(16):
        oh16[r, r * B:(r + 1) * B] = 1.0
    oh16 = oh16.astype(bf)

    maps = []
    for c in range(NCORES):
        bs = slice(c * B, (c + 1) * B)
        xs = x[bs]            # [16, T]
        ln = lengths[bs]      # [16]
        tg = tags[bs]         # [16, T]
        m = {"trans": trans, "transT": trans.T.astype(f32).copy(), "fcb": fcb,
             "onehot16": oh16}
        # host embedding gather (pure indexing): embT[p, k*T*B + t*16 + b]
        xe = emb[xs]                                  # [16, T, E]
        m["xembT"] = np.ascontiguousarray(
            xe.transpose(2, 1, 0).reshape(2, P, T * B)
        ).reshape(E, T * B).astype(bf)
        for d in ("f", "b"):
            m[f"wihT_{d}"], m[f"whhT_{d}"] = Wd[d]
            m[f"bias16_{d}"] = Bd[d]
            m[f"fcWT_{d}"] = fcWT[d]
            di = 0 if d == "f" else 1
            h0T = h0[di, bs].T.reshape(4, P, B).transpose(1, 0, 2).reshape(P, 64)
            c0T = c0[di, bs].T.reshape(4, P, B).transpose(1, 0, 2).reshape(P, 64)
            m[f"h0T_{d}"] = h0T.astype(bf).copy()
            m[f"c0T_{d}"] = c0T.astype(f32).copy()
        # bwd mask: step s processes tau = T-1-s; valid iff tau < len
        tau = (T - 1 - np.arange(T))[:, None]          # [T, 1]
        mk = (tau < ln[None, :]).astype(f32)           # [T, 16]
        m["mask_b"] = np.broadcast_to(
            mk[:, None, None, :], (T, P, 4, B)).reshape(T, P, 64).astype(np.uint8).copy()
        a0 = np.zeros((K, B), f32); a0[START, :] = 1.0
        m["a0"] = a0
        msel = np.zeros((K, T, B), f32)
        msel[:, ln - 1, np.arange(B)] = 1.0
        m["msel"] = msel.reshape(K, T * B)
        mep = np.zeros((NE, B), f32)
        mep[(ln - 1) // R, np.arange(B)] = 1.0
        m["maskep"] = mep.reshape(-1)
        tarange = np.arange(T)[None, :]
        valid = tarange < ln[:, None]                  # [16, T]
        selm = np.zeros((K, T, B), f32)
        jj = np.arange(K)[:, None, None]
        selm[:] = (tg.T[None] == jj) & valid.T[None]
        m["sel"] = np.ascontiguousarray(selm.reshape(K, T * B))
        counts = np.zeros((B, 144), f32)
        cntb = np.zeros((B, K), f32)
        for b in range(B):
            L = int(ln[b])
            prev = START
            for t in range(L):
                nx = int(tg[b, t])
                counts[b, nx * K + prev] += 1
                cntb[b, nx] += 1
                prev = nx
            counts[b, STOP * K + prev] += 1
        m["counts"] = counts
        m["cntb"] = cntb
        maps.append(m)
    return maps


def kernel(**inputs):
    from concourse.bass_utils import run_bass_kernel_spmd
    nc = get_program()
    maps = host_prep(inputs)
    res = run_bass_kernel_spmd(nc, maps, core_ids=list(range(NCORES)))
    out = np.concatenate([r["nll"] for r in res.results]).astype(np.float32)
    kernel.last_results = res
    return out
